# revision 13
# baseline (speedup 1.0000x reference)
"""Trainium2 Bass kernel for nn_EngramModule (scatter_memory).

Computation (see reference): hashed-embedding gather -> value/key projections
-> RMSNorm q/k -> sqrt-sigmoid gate -> gated value -> depthwise causal conv
(K=4) -> silu residual.

Sharding: 8 cores, data-parallel over (batch, sequence-half): core c handles
batch c//2, tokens [1024*2*(c%2) ... +2048). The conv needs only 3 halo
tokens, which each core recomputes locally (zero-padded via an appended
zero row of the embedding table for the first half). The embedding table is
replicated on every core (204 MB), so no collectives at all.

Per-core pipeline (T = 2176 = 3 halo + 2048 real + 125 pad tokens, 17 tiles
of 128 tokens):
  pass 1 (token-major): per tile: gather 8 embeddings/token (indirect DMA),
    PE-transpose mem -> mem_T (e-major, fp32r), key matmuls (fp32r) into
    PSUM, fused reductions: qq = sum q^2 (DVE stt+accum), kk = sum k^2
    (ACT Square+accum from PSUM), qk = sum q*k (DVE stt+accum, PSUM operand).
  bulk gate: score = 32*qk/sqrt((qq+D*eps)(kk+D*eps)); z = sign(s)*sqrt(|s|)
    via sqrt ops; gate = 0.5 + 0.5*tanh(z/2) (NaN-free sigmoid); transposed to
    a [1, T] row and partition-broadcast to [128, T] per head.
  pass 2 (channel-major): value_T = vw^T @ mem_T (fp32r, PSUM), gated_T =
    value_T * gate_bc (bf16), depthwise conv as 4 free-dim-shifted
    tensor_scalar/stt taps with per-partition channel weights, silu (+conv
    bias) on ACT, residual add, PE-transpose back to token-major, DMA out
    with bf16->f32 cast.
"""

import os
import sys

sys.path.insert(0, "/opt/trn_rl_repo")

import numpy as np
from contextlib import ExitStack

import jax

jax.config.update("jax_compilation_cache_dir", "/tmp/jax_neff_cache")
jax.config.update("jax_persistent_cache_min_entry_size_bytes", -1)
jax.config.update("jax_persistent_cache_min_compile_time_secs", 0.0)

import concourse.bass as bass
import concourse.bacc as bacc
import concourse.tile as tile
from concourse import mybir
from concourse.masks import make_identity
from concourse.bass_utils import run_bass_kernel_spmd

f32 = mybir.dt.float32
f32r = mybir.dt.float32r
bf16 = mybir.dt.bfloat16
i32 = mybir.dt.int32
AF = mybir.ActivationFunctionType
OP = mybir.AluOpType

# problem constants
B, L, HC, D = 4, 4096, 2, 1024
NH, ED = 8, 64
VOCAB = 100000
K = 4
E = NH * ED  # 512
EPS = 1e-6
C = HC * D  # 2048
P = 128

N_CORES = 8
TOK_PER_CORE = (B * L) // N_CORES  # 2048
HALO = K - 1  # 3


class Cfg:
    """Geometry for one core. full=True is the real problem size."""

    def __init__(self, full=True):
        if full:
            self.n_tiles = 17          # token tiles of 128
            self.out_tok = 2048        # real output tokens per core
        else:
            self.n_tiles = 3           # small config for validation
            self.out_tok = 256
        self.T = self.n_tiles * P      # padded token space
        # pass-2 halves: each produces out_tok/2 output tokens
        self.half_out = self.out_tok // 2            # 1024 (full)
        assert self.half_out % P == 0
        self.n_blk = self.half_out // P              # token blocks per half
        # gated_T cols needed per half: half_out + HALO, in matmul windows of
        # <=512 whose starts stay 128-aligned (full windows are multiples of
        # 128; only the last window may be a small remainder)
        need = self.half_out + HALO + 1   # +1: fp32r matmuls need even N
        ws = []
        left = need
        while left > 0:
            w = min(512, (left // P) * P) if left > P else left
            ws.append(w)
            left -= w
        self.half_windows = ws                        # e.g. [512, 512, 3]
        self.half_cols = sum(ws)                      # cols of gated per half


_CACHE = {}


def _build(cfg: Cfg, has_kb, has_vb, has_wqk):
    nc = bacc.Bacc("TRN2", target_bir_lowering=False, debug=False)
    T, NT = cfg.T, cfg.n_tiles
    NCOL = 2 * NT  # score columns (tile, head)

    hid_d = nc.dram_tensor("hid", [T, C], f32, kind="ExternalInput")
    ids_d = nc.dram_tensor("idsb", [P, NT * NH], i32, kind="ExternalInput")
    emb_d = nc.dram_tensor("emb", [VOCAB * NH + 8, ED], f32, kind="ExternalInput")
    vw_d = nc.dram_tensor("vw", [P, 4, D], f32, kind="ExternalInput")
    kw_d = nc.dram_tensor("kw", [P, HC, 4, D], f32, kind="ExternalInput")
    wconv_d = nc.dram_tensor("wconv", [P, 16, K], f32, kind="ExternalInput")
    cb_d = nc.dram_tensor("cb", [P, 16], f32, kind="ExternalInput")
    gstage_d = nc.dram_tensor("gstage", [NCOL, P], f32)  # internal staging
    if has_wqk:
        wqk_d = nc.dram_tensor("wqkb", [P, C], f32, kind="ExternalInput")
    if has_kb:
        kb_d = nc.dram_tensor("kb", [1, HC, D], f32, kind="ExternalInput")
    if has_vb:
        vb_d = nc.dram_tensor("vb", [1, D], f32, kind="ExternalInput")
    out_d = nc.dram_tensor("out", [cfg.out_tok, C], f32, kind="ExternalOutput")

    with tile.TileContext(nc) as tc:
        with ExitStack() as ctx:
            # ---- persistent pools ----
            pers = ctx.enter_context(tc.tile_pool(name="pers", bufs=1))

            ident = pers.tile([P, P], f32)
            make_identity(nc, ident[:])
            ident_b = pers.tile([P, P], bf16)
            nc.vector.tensor_copy(ident_b[:], ident[:])
            ident_r = pers.tile([P, P], f32r)
            nc.vector.tensor_copy(ident_r[:], ident[:])

            vw_r = pers.tile([P, 4, D], f32r)
            kw_r = pers.tile([P, HC, 4, D], f32r)
            kb_r = pers.tile([1, HC, D], f32r) if has_kb else None
            vb_r = pers.tile([1, D], f32r) if has_vb else None
            ones1_r = pers.tile([1, max(P, 512)], f32r) if (has_kb or has_vb) else None
            # weights -> SBUF (f32 staging, freed before the main passes) -> f32r
            with ExitStack() as cs:
                stage = cs.enter_context(tc.tile_pool(name="stage", bufs=1))
                vw_s = stage.tile([P, 4, D], f32)
                nc.sync.dma_start(vw_s[:], vw_d[:, :, :])
                nc.vector.tensor_copy(vw_r[:], vw_s[:])
                kw_s = stage.tile([P, HC, 4, D], f32)
                nc.sync.dma_start(kw_s[:], kw_d[:, :, :, :])
                nc.vector.tensor_copy(kw_r[:], kw_s[:])
                if has_kb:
                    kb_s = stage.tile([1, HC, D], f32)
                    nc.sync.dma_start(kb_s[:], kb_d[:, :, :])
                    nc.vector.tensor_copy(kb_r[:], kb_s[:])
                if has_vb:
                    vb_s = stage.tile([1, D], f32)
                    nc.sync.dma_start(vb_s[:], vb_d[:, :])
                    nc.vector.tensor_copy(vb_r[:], vb_s[:])
                if has_kb or has_vb:
                    ones1_s = stage.tile([1, max(P, 512)], f32)
                    nc.vector.memset(ones1_s[:], 1.0)
                    nc.vector.tensor_copy(ones1_r[:], ones1_s[:])
            wconv_sb = pers.tile([P, 16, K], f32)
            nc.sync.dma_start(wconv_sb[:], wconv_d[:, :, :])
            cb_sb = pers.tile([P, 16], f32)
            nc.sync.dma_start(cb_sb[:], cb_d[:, :])
            ids_sb = pers.tile([P, NT * NH], i32)
            nc.sync.dma_start(ids_sb[:], ids_d[:, :])
            if has_wqk:
                wqk_sb = pers.tile([P, C], bf16)
                nc.gpsimd.dma_start(wqk_sb[:], wqk_d[:, :])

            # mem_T: e-major gathered embeddings, fp32r [P(e), tile, echunk, t]
            mem_T = pers.tile([P, NT, 4, P], f32r)
            # score stats
            qq_all = pers.tile([P, NCOL], f32)
            kk_all = pers.tile([P, NCOL], f32)
            qk_all = pers.tile([P, NCOL], f32)
            gate_bc = pers.tile([P, HC, T], f32)

            # ================= PASS 1 =================
            with ExitStack() as c1:
                p1 = c1.enter_context(tc.tile_pool(name="p1", bufs=2))
                p1ps = c1.enter_context(tc.tile_pool(name="p1ps", bufs=2, space="PSUM"))
                dve_dummy = pers.tile([P, 1], f32)
                for tb in range(NT):
                    hid_b = p1.tile([P, C], f32, tag="hid")
                    nc.sync.dma_start(hid_b[:], hid_d[tb * P:(tb + 1) * P, :])
                    mem = p1.tile([P, NH, ED], f32, tag="mem")
                    for j in range(NH):
                        nc.gpsimd.indirect_dma_start(
                            out=mem[:, j, :],
                            out_offset=None,
                            in_=emb_d[:, :],
                            in_offset=bass.IndirectOffsetOnAxis(
                                ap=ids_sb[:, tb * NH + j: tb * NH + j + 1], axis=0
                            ),
                        )
                    # round to f32r, then transpose mem -> mem_T
                    # (fp32 transpose_mode trips s3d3_mm_fp32r_restrictions;
                    # f32r is full-speed and matches the matmul rounding)
                    mem_r = p1.tile([P, NH, ED], f32r, tag="memr")
                    nc.vector.tensor_copy(mem_r[:], mem[:])
                    tp = p1ps.tile([P, 4, P], f32r, tag="tp", space="PSUM")
                    for k in range(4):
                        nc.tensor.transpose(
                            out=tp[:, k, :],
                            in_=mem_r[:, 2 * k: 2 * k + 2, :],
                            identity=ident_r[:],
                        )
                    nc.vector.tensor_copy(mem_T[:, tb, :, :], tp[:, :, :])
                    # key matmuls + score reductions, per head
                    for h in range(HC):
                        kps = p1ps.tile([P, D], f32, tag="kps", space="PSUM")
                        for k in range(4):
                            for n in range(2):
                                nc.tensor.matmul(
                                    out=kps[:, n * 512:(n + 1) * 512],
                                    lhsT=mem_T[:, tb, k, :],
                                    rhs=kw_r[:, h, k, n * 512:(n + 1) * 512],
                                    start=(k == 0),
                                    stop=(k == 3) and not has_kb,
                                )
                        if has_kb:
                            for n in range(2):
                                nc.tensor.matmul(
                                    out=kps[:, n * 512:(n + 1) * 512],
                                    lhsT=ones1_r[:, 0:P],
                                    rhs=kb_r[:, h, n * 512:(n + 1) * 512],
                                    start=False,
                                    stop=True,
                                )
                        col = 2 * tb + h
                        junk = p1.tile([P, D], bf16, tag="actjunk")
                        nc.scalar.activation(
                            out=junk[:],
                            in_=kps[:, :],
                            func=AF.Square,
                            accum_out=kk_all[:, col: col + 1],
                        )
                        qsrc = hid_b[:, h * D:(h + 1) * D]
                        if has_wqk:
                            hw = p1.tile([P, D], f32, tag="hidw")
                            nc.vector.tensor_tensor(
                                out=hw[:], in0=qsrc, in1=wqk_sb[:, h * D:(h + 1) * D],
                                op=OP.mult,
                            )
                            qk_in0 = hw[:]
                        else:
                            qk_in0 = qsrc
                        nc.vector.scalar_tensor_tensor(
                            out=dve_dummy[:].broadcast_to([P, D]),
                            in0=qk_in0,
                            scalar=1.0,
                            in1=kps[:, :],
                            op0=OP.bypass,
                            op1=OP.mult,
                            accum_out=qk_all[:, col: col + 1],
                        )
                        nc.vector.scalar_tensor_tensor(
                            out=dve_dummy[:].broadcast_to([P, D]),
                            in0=qsrc,
                            scalar=1.0,
                            in1=qsrc,
                            op0=OP.bypass,
                            op1=OP.mult,
                            accum_out=qq_all[:, col: col + 1],
                        )

            # ================= BULK GATE =================
            with ExitStack() as c2:
                bg = c2.enter_context(tc.tile_pool(name="bg", bufs=1))
                bgps = c2.enter_context(tc.tile_pool(name="bgps", bufs=1, space="PSUM"))
                DE = float(D) * EPS
                aq = bg.tile([P, NCOL], f32)
                # (qq+De)/1024  -> folds the sqrt(D) scale into the denom
                nc.vector.tensor_scalar(aq[:], qq_all[:], DE, 1.0 / D, OP.add, OP.mult)
                ak = bg.tile([P, NCOL], f32)
                nc.vector.tensor_scalar(ak[:], kk_all[:], DE, None, OP.add)
                u = bg.tile([P, NCOL], f32)
                nc.vector.tensor_tensor(out=u[:], in0=aq[:], in1=ak[:], op=OP.mult)
                sden = bg.tile([P, NCOL], f32)
                nc.scalar.activation(sden[:], u[:], AF.Sqrt)
                rden = bg.tile([P, NCOL], f32)
                nc.vector.reciprocal(rden[:], sden[:])
                s = bg.tile([P, NCOL], f32)
                nc.vector.tensor_tensor(out=s[:], in0=qk_all[:], in1=rden[:], op=OP.mult)
                s2 = bg.tile([P, NCOL], f32)
                nc.vector.tensor_tensor(out=s2[:], in0=s[:], in1=s[:], op=OP.mult)
                nc.vector.tensor_scalar_max(s2[:], s2[:], 1e-12)
                sab = bg.tile([P, NCOL], f32)
                nc.scalar.activation(sab[:], s2[:], AF.Sqrt)      # = max(|s|,1e-6)
                q4 = bg.tile([P, NCOL], f32)
                nc.scalar.activation(q4[:], sab[:], AF.Sqrt)      # = |s|_c^(1/2)
                rq4 = bg.tile([P, NCOL], f32)
                nc.vector.reciprocal(rq4[:], q4[:])
                z = bg.tile([P, NCOL], f32)
                nc.vector.tensor_tensor(out=z[:], in0=s[:], in1=rq4[:], op=OP.mult)
                th = bg.tile([P, NCOL], f32)
                nc.scalar.activation(th[:], z[:], AF.Tanh, scale=0.5)
                gate = bg.tile([P, NCOL], f32)
                nc.vector.tensor_scalar(gate[:], th[:], 0.5, 0.5, OP.mult, OP.add)
                # transpose gate [P, NCOL] -> [NCOL, P] and lay out per head
                gate_r = bg.tile([P, P], f32r)
                nc.vector.memset(gate_r[:].bitcast(f32), 0.0)
                nc.vector.tensor_copy(gate_r[:, 0:NCOL], gate[:])
                gps = bgps.tile([P, P], f32r, space="PSUM")
                nc.tensor.transpose(
                    out=gps[:, :], in_=gate_r[:, :], identity=ident_r[:]
                )
                gT = bg.tile([NCOL, P], f32)
                nc.vector.tensor_copy(gT[:, :], gps[0:NCOL, :])
                nc.sync.dma_start(gstage_d[:, :], gT[:, :])
                for h in range(HC):
                    glin = bg.tile([1, T], f32, tag=f"glin{h}")
                    nc.sync.dma_start(
                        glin[0:1, :].rearrange("h (t p) -> h t p", p=P),
                        gstage_d[:, :].rearrange("(t h) p -> h t p", h=HC)[
                            h: h + 1, :, :
                        ],
                    )
                    nc.gpsimd.partition_broadcast(gate_bc[:, h, :], glin[0:1, :])

            # ================= PASS 2 =================
            for half in range(2):
                g0 = half * cfg.half_out  # first gated token col of this half
                with ExitStack() as c3:
                    p2 = c3.enter_context(tc.tile_pool(name=f"p2_{half}", bufs=1))
                    accp = c3.enter_context(tc.tile_pool(name=f"acc_{half}", bufs=2))
                    p2ps = c3.enter_context(
                        tc.tile_pool(name=f"p2ps_{half}", bufs=3, space="PSUM")
                    )
                    outps = c3.enter_context(
                        tc.tile_pool(name=f"ops_{half}", bufs=2, space="PSUM")
                    )
                    HCOL = cfg.half_cols
                    gated = p2.tile([P, HC, 8, HCOL], bf16)

                    def mem_window(k, t0, wN):
                        # e-chunk k of mem_T over tokens [t0, t0+wN); t0 is
                        # always 128-aligned, wN is a multiple of 128 or fits
                        # within one tile.
                        assert t0 % P == 0
                        tb0 = t0 // P
                        if wN % P == 0:
                            return mem_T[:, tb0: tb0 + wN // P, k, :]
                        assert wN < P
                        return mem_T[:, tb0, k, 0:wN]

                    for dc in range(8):
                        wof = 0
                        for wN in cfg.half_windows:
                            vps = p2ps.tile([P, 512], f32, tag="vps", space="PSUM")
                            # token columns g0+wof .. g0+wof+wN of mem_T
                            t0 = g0 + wof
                            for k in range(4):
                                nc.tensor.matmul(
                                    out=vps[:, 0:wN],
                                    lhsT=vw_r[:, k, dc * P:(dc + 1) * P],
                                    rhs=mem_window(k, t0, wN),
                                    start=(k == 0),
                                    stop=(k == 3) and not has_vb,
                                )
                            if has_vb:
                                nc.tensor.matmul(
                                    out=vps[:, 0:wN],
                                    lhsT=vb_r[:, dc * P:(dc + 1) * P],
                                    rhs=ones1_r[:, 0:wN],
                                    start=False,
                                    stop=True,
                                )
                            for h in range(HC):
                                nc.vector.tensor_tensor(
                                    out=gated[:, h, dc, wof: wof + wN],
                                    in0=vps[:, 0:wN],
                                    in1=gate_bc[:, h, t0: t0 + wN],
                                    op=OP.mult,
                                )
                            wof += wN
                    # conv + silu + residual per channel chunk
                    HO = cfg.half_out
                    ocs = []
                    for h in range(HC):
                        for dc in range(8):
                            cc = h * 8 + dc
                            g = gated[:, h, dc, :]
                            a1 = accp.tile([P, HO], bf16, tag="acc")
                            nc.vector.tensor_scalar_mul(
                                a1[:], g[0:P, 0:HO], wconv_sb[:, cc, 0:1]
                            )
                            prev = a1
                            for j in range(1, K):
                                aj = accp.tile([P, HO], bf16, tag="acc")
                                nc.vector.scalar_tensor_tensor(
                                    out=aj[:],
                                    in0=g[0:P, j: j + HO],
                                    scalar=wconv_sb[:, cc, j: j + 1],
                                    in1=prev[:],
                                    op0=OP.mult,
                                    op1=OP.add,
                                )
                                prev = aj
                            ysil = accp.tile([P, HO], bf16, tag="ysil")
                            nc.scalar.activation(
                                out=ysil[:],
                                in_=prev[:],
                                func=AF.Silu,
                                bias=cb_sb[:, cc: cc + 1],
                            )
                            oc = p2.tile([P, HO], bf16, tag=f"oc{cc}")
                            nc.vector.tensor_tensor(
                                out=oc[:], in0=g[0:P, HALO: HALO + HO],
                                in1=ysil[:], op=OP.add,
                            )
                            ocs.append(oc)
                    # transpose back to token-major + store, per token block
                    for m in range(cfg.n_blk):
                        ops_t = outps.tile([P, 16, P], bf16, tag="outps",
                                           space="PSUM")
                        for cc in range(16):
                            nc.tensor.transpose(
                                out=ops_t[:, cc, :],
                                in_=ocs[cc][:, m * P:(m + 1) * P],
                                identity=ident_b[:],
                            )
                        osb = accp.tile([P, C], bf16, tag="osb")
                        nc.vector.tensor_copy(osb[:], ops_t[:, :, :])
                        r0 = half * cfg.half_out + m * P
                        nc.gpsimd.dma_start(out_d[r0: r0 + P, :], osb[:])

    nc.compile()
    return nc


def _prep_core_inputs(cfg, core, flat_ids, hid_flat, shared):
    """Slice per-core inputs. flat_ids [B, L, NH] i32, hid_flat [B, L, C] f32."""
    T, NT = cfg.T, cfg.n_tiles
    b = core // 2
    half = core % 2
    start = half * cfg.out_tok
    ZID = VOCAB * NH  # zero row

    ids_core = np.full((T, NH), ZID, dtype=np.int32)
    hid_core = np.zeros((T, C), dtype=np.float32)
    ids_core[HALO: HALO + cfg.out_tok] = flat_ids[b, start: start + cfg.out_tok]
    hid_core[HALO: HALO + cfg.out_tok] = hid_flat[b, start: start + cfg.out_tok]
    if start >= HALO:
        ids_core[0:HALO] = flat_ids[b, start - HALO: start]
        hid_core[0:HALO] = hid_flat[b, start - HALO: start]

    idsb = np.ascontiguousarray(
        ids_core.reshape(NT, P, NH).transpose(1, 0, 2).reshape(P, NT * NH)
    )
    m = dict(shared)
    m["hid"] = hid_core
    m["idsb"] = idsb
    return m


def kernel(hidden_states, hash_ids, offsets, emb_table, value_w, value_b,
           key_w, key_b, norm_key_w, norm_query_w, conv_w, conv_b):
    cfg = Cfg(full=True)
    return _run(cfg, hidden_states, hash_ids, offsets, emb_table, value_w,
                value_b, key_w, key_b, norm_key_w, norm_query_w, conv_w,
                conv_b)


def _run(cfg, hidden_states, hash_ids, offsets, emb_table, value_w, value_b,
         key_w, key_b, norm_key_w, norm_query_w, conv_w, conv_b,
         trace=False):
    hidden_states = np.asarray(hidden_states)
    hash_ids = np.asarray(hash_ids, dtype=np.int32)
    offsets = np.asarray(offsets, dtype=np.int32)
    emb_table = np.asarray(emb_table, dtype=np.float32)

    wqk = (np.asarray(norm_query_w) * np.asarray(norm_key_w)).astype(np.float32)
    has_wqk = not np.allclose(wqk, 1.0)
    has_kb = not np.all(np.asarray(key_b) == 0.0)
    has_vb = not np.all(np.asarray(value_b) == 0.0)

    key = (cfg.n_tiles, has_kb, has_vb, has_wqk)
    if key not in _CACHE:
        _CACHE[key] = _build(cfg, has_kb, has_vb, has_wqk)
    nc = _CACHE[key]

    # shared (replicated) input arrays
    emb_aug = np.zeros((VOCAB * NH + 8, ED), dtype=np.float32)
    emb_aug[: VOCAB * NH] = emb_table
    vw_h = np.ascontiguousarray(
        np.asarray(value_w, dtype=np.float32).reshape(4, P, D).transpose(1, 0, 2)
    )
    kw_h = np.ascontiguousarray(
        np.asarray(key_w, dtype=np.float32)
        .reshape(HC, 4, P, D)
        .transpose(2, 0, 1, 3)
    )
    wc = np.asarray(conv_w, dtype=np.float32)[:, 0, :]  # [C, K]
    wconv_h = np.ascontiguousarray(wc.reshape(16, P, K).transpose(1, 0, 2))
    cb_h = np.ascontiguousarray(
        np.asarray(conv_b, dtype=np.float32).reshape(16, P).T
    )
    shared = dict(emb=emb_aug, vw=vw_h, kw=kw_h, wconv=wconv_h, cb=cb_h)
    if has_wqk:
        shared["wqkb"] = np.ascontiguousarray(
            np.broadcast_to(wqk.reshape(1, C), (P, C))
        ).astype(np.float32)
    if has_kb:
        shared["kb"] = np.asarray(key_b, dtype=np.float32).reshape(1, HC, D)
    if has_vb:
        shared["vb"] = np.asarray(value_b, dtype=np.float32).reshape(1, D)

    flat_ids = hash_ids + offsets[None, None, :]
    hid_flat = np.ascontiguousarray(
        hidden_states.reshape(B, L, C), dtype=np.float32
    )

    in_maps = [
        _prep_core_inputs(cfg, c, flat_ids, hid_flat, shared)
        for c in range(N_CORES)
    ]
    res = run_bass_kernel_spmd(
        nc, in_maps, core_ids=list(range(N_CORES)), trace=trace
    )

    out = np.empty((B, L, HC, D), dtype=np.float32)
    for c in range(N_CORES):
        b, half = c // 2, c % 2
        start = half * cfg.out_tok
        out[b, start: start + cfg.out_tok] = res.results[c]["out"].reshape(
            cfg.out_tok, HC, D
        )
    if trace:
        return out, res
    return out


# revision 14
# speedup vs baseline: 1.0173x; 1.0173x over previous
"""Trainium2 Bass kernel for nn_EngramModule (scatter_memory).

Computation (see reference): hashed-embedding gather -> value/key projections
-> RMSNorm q/k -> sqrt-sigmoid gate -> gated value -> depthwise causal conv
(K=4) -> silu residual.

Sharding: 8 cores, data-parallel over (batch, sequence-half): core c handles
batch c//2, tokens [1024*2*(c%2) ... +2048). The conv needs only 3 halo
tokens, which each core recomputes locally (zero-padded via an appended
zero row of the embedding table for the first half). The embedding table is
replicated on every core (204 MB), so no collectives at all.

Per-core pipeline (T = 2176 = 3 halo + 2048 real + 125 pad tokens, 17 tiles
of 128 tokens):
  pass 1 (token-major): per tile: gather 8 embeddings/token (indirect DMA),
    PE-transpose mem -> mem_T (e-major, fp32r), key matmuls (fp32r) into
    PSUM, fused reductions: qq = sum q^2 (DVE stt+accum), kk = sum k^2
    (ACT Square+accum from PSUM), qk = sum q*k (DVE stt+accum, PSUM operand).
  bulk gate: score = 32*qk/sqrt((qq+D*eps)(kk+D*eps)); z = sign(s)*sqrt(|s|)
    via sqrt ops; gate = 0.5 + 0.5*tanh(z/2) (NaN-free sigmoid); transposed to
    a [1, T] row and partition-broadcast to [128, T] per head.
  pass 2 (channel-major): value_T = vw^T @ mem_T (fp32r, PSUM), gated_T =
    value_T * gate_bc (bf16), depthwise conv as 4 free-dim-shifted
    tensor_scalar/stt taps with per-partition channel weights, silu (+conv
    bias) on ACT, residual add, PE-transpose back to token-major, DMA out
    with bf16->f32 cast.
"""

import os
import sys

sys.path.insert(0, "/opt/trn_rl_repo")

import numpy as np
from contextlib import ExitStack

import jax

jax.config.update("jax_compilation_cache_dir", "/tmp/jax_neff_cache")
jax.config.update("jax_persistent_cache_min_entry_size_bytes", -1)
jax.config.update("jax_persistent_cache_min_compile_time_secs", 0.0)

import concourse.bass as bass
import concourse.bacc as bacc
import concourse.tile as tile
from concourse import mybir
from concourse.masks import make_identity
from concourse.bass_utils import run_bass_kernel_spmd

f32 = mybir.dt.float32
f32r = mybir.dt.float32r
bf16 = mybir.dt.bfloat16
i32 = mybir.dt.int32
AF = mybir.ActivationFunctionType
OP = mybir.AluOpType

# problem constants
B, L, HC, D = 4, 4096, 2, 1024
NH, ED = 8, 64
VOCAB = 100000
K = 4
E = NH * ED  # 512
EPS = 1e-6
C = HC * D  # 2048
P = 128

N_CORES = 8
TOK_PER_CORE = (B * L) // N_CORES  # 2048
HALO = K - 1  # 3


class Cfg:
    """Geometry for one core. full=True is the real problem size."""

    def __init__(self, full=True):
        if full:
            self.n_tiles = 17          # token tiles of 128
            self.out_tok = 2048        # real output tokens per core
        else:
            self.n_tiles = 3           # small config for validation
            self.out_tok = 256
        self.T = self.n_tiles * P      # padded token space
        # pass-2 halves: each produces out_tok/2 output tokens
        self.half_out = self.out_tok // 2            # 1024 (full)
        assert self.half_out % P == 0
        self.n_blk = self.half_out // P              # token blocks per half
        # gated_T cols needed per half: half_out + HALO, in matmul windows of
        # <=512 whose starts stay 128-aligned (full windows are multiples of
        # 128; only the last window may be a small remainder)
        need = self.half_out + HALO + 1   # +1: fp32r matmuls need even N
        ws = []
        left = need
        while left > 0:
            w = min(512, (left // P) * P) if left > P else left
            ws.append(w)
            left -= w
        self.half_windows = ws                        # e.g. [512, 512, 3]
        self.half_cols = sum(ws)                      # cols of gated per half


_CACHE = {}


def _build(cfg: Cfg, has_kb, has_vb, has_wqk):
    nc = bacc.Bacc("TRN2", target_bir_lowering=False, debug=False)
    T, NT = cfg.T, cfg.n_tiles
    NCOL = 2 * NT  # score columns (tile, head)

    hid_d = nc.dram_tensor("hid", [T, C], f32, kind="ExternalInput")
    ids_d = nc.dram_tensor("idsb", [P, NT * NH], i32, kind="ExternalInput")
    emb_d = nc.dram_tensor("emb", [VOCAB * NH + 8, ED], f32, kind="ExternalInput")
    vw_d = nc.dram_tensor("vw", [P, 4, D], f32, kind="ExternalInput")
    kw_d = nc.dram_tensor("kw", [P, HC, 4, D], f32, kind="ExternalInput")
    wconv_d = nc.dram_tensor("wconv", [P, 16, K], f32, kind="ExternalInput")
    cb_d = nc.dram_tensor("cb", [P, 16], f32, kind="ExternalInput")
    gstage_d = nc.dram_tensor("gstage", [NCOL, P], f32)  # internal staging
    if has_wqk:
        wqk_d = nc.dram_tensor("wqkb", [P, C], f32, kind="ExternalInput")
    if has_kb:
        kb_d = nc.dram_tensor("kb", [1, HC, D], f32, kind="ExternalInput")
    if has_vb:
        vb_d = nc.dram_tensor("vb", [1, D], f32, kind="ExternalInput")
    out_d = nc.dram_tensor("out", [cfg.out_tok, C], f32, kind="ExternalOutput")

    with tile.TileContext(nc) as tc:
        with ExitStack() as ctx:
            # ---- persistent pools ----
            pers = ctx.enter_context(tc.tile_pool(name="pers", bufs=1))

            ident = pers.tile([P, P], f32)
            make_identity(nc, ident[:])
            ident_b = pers.tile([P, P], bf16)
            nc.vector.tensor_copy(ident_b[:], ident[:])
            ident_r = pers.tile([P, P], f32r)
            nc.vector.tensor_copy(ident_r[:], ident[:])

            vw_r = pers.tile([P, 4, D], f32r)
            kw_r = pers.tile([P, HC, 4, D], f32r)
            kb_r = pers.tile([1, HC, D], f32r) if has_kb else None
            vb_r = pers.tile([1, D], f32r) if has_vb else None
            ones1_r = pers.tile([1, max(P, 512)], f32r) if (has_kb or has_vb) else None
            # weights -> SBUF (f32 staging, freed before the main passes) -> f32r
            with ExitStack() as cs:
                stage = cs.enter_context(tc.tile_pool(name="stage", bufs=1))
                vw_s = stage.tile([P, 4, D], f32)
                nc.sync.dma_start(vw_s[:], vw_d[:, :, :])
                nc.vector.tensor_copy(vw_r[:], vw_s[:])
                kw_s = stage.tile([P, HC, 4, D], f32)
                nc.sync.dma_start(kw_s[:], kw_d[:, :, :, :])
                nc.vector.tensor_copy(kw_r[:], kw_s[:])
                if has_kb:
                    kb_s = stage.tile([1, HC, D], f32)
                    nc.sync.dma_start(kb_s[:], kb_d[:, :, :])
                    nc.vector.tensor_copy(kb_r[:], kb_s[:])
                if has_vb:
                    vb_s = stage.tile([1, D], f32)
                    nc.sync.dma_start(vb_s[:], vb_d[:, :])
                    nc.vector.tensor_copy(vb_r[:], vb_s[:])
                if has_kb or has_vb:
                    ones1_s = stage.tile([1, max(P, 512)], f32)
                    nc.vector.memset(ones1_s[:], 1.0)
                    nc.vector.tensor_copy(ones1_r[:], ones1_s[:])
            wconv_sb = pers.tile([P, 16, K], f32)
            nc.sync.dma_start(wconv_sb[:], wconv_d[:, :, :])
            cb_sb = pers.tile([P, 16], f32)
            nc.sync.dma_start(cb_sb[:], cb_d[:, :])
            ids_sb = pers.tile([P, NT * NH], i32)
            nc.sync.dma_start(ids_sb[:], ids_d[:, :])
            if has_wqk:
                wqk_sb = pers.tile([P, C], bf16)
                nc.gpsimd.dma_start(wqk_sb[:], wqk_d[:, :])

            # mem_T: e-major gathered embeddings, fp32r [P(e), tile, echunk, t]
            mem_T = pers.tile([P, NT, 4, P], f32r)
            # score stats
            qq_all = pers.tile([P, NCOL], f32)
            kk_all = pers.tile([P, NCOL], f32)
            qk_all = pers.tile([P, NCOL], f32)
            gate_bc = pers.tile([P, HC, T], f32)

            # ================= PASS 1 =================
            with ExitStack() as c1:
                p1 = c1.enter_context(tc.tile_pool(name="p1", bufs=4))
                p1ps = c1.enter_context(tc.tile_pool(name="p1ps", bufs=2, space="PSUM"))
                dve_dummy = pers.tile([P, 1], f32)
                for tb in range(NT):
                    hid_b = p1.tile([P, C], f32, tag="hid")
                    nc.sync.dma_start(hid_b[:], hid_d[tb * P:(tb + 1) * P, :])
                    mem = p1.tile([P, NH, ED], f32, tag="mem")
                    for j in range(NH):
                        nc.gpsimd.indirect_dma_start(
                            out=mem[:, j, :],
                            out_offset=None,
                            in_=emb_d[:, :],
                            in_offset=bass.IndirectOffsetOnAxis(
                                ap=ids_sb[:, tb * NH + j: tb * NH + j + 1], axis=0
                            ),
                        )
                    # round to f32r, then transpose mem -> mem_T
                    # (fp32 transpose_mode trips s3d3_mm_fp32r_restrictions;
                    # f32r is full-speed and matches the matmul rounding)
                    mem_r = p1.tile([P, NH, ED], f32r, tag="memr")
                    nc.vector.tensor_copy(mem_r[:], mem[:])
                    tp = p1ps.tile([P, 4, P], f32r, tag="tp", space="PSUM")
                    for k in range(4):
                        nc.tensor.transpose(
                            out=tp[:, k, :],
                            in_=mem_r[:, 2 * k: 2 * k + 2, :],
                            identity=ident_r[:],
                        )
                    nc.vector.tensor_copy(mem_T[:, tb, :, :], tp[:, :, :])
                    # key matmuls + score reductions, per head
                    for h in range(HC):
                        kps = p1ps.tile([P, D], f32, tag="kps", space="PSUM")
                        for k in range(4):
                            for n in range(2):
                                nc.tensor.matmul(
                                    out=kps[:, n * 512:(n + 1) * 512],
                                    lhsT=mem_T[:, tb, k, :],
                                    rhs=kw_r[:, h, k, n * 512:(n + 1) * 512],
                                    start=(k == 0),
                                    stop=(k == 3) and not has_kb,
                                )
                        if has_kb:
                            for n in range(2):
                                nc.tensor.matmul(
                                    out=kps[:, n * 512:(n + 1) * 512],
                                    lhsT=ones1_r[:, 0:P],
                                    rhs=kb_r[:, h, n * 512:(n + 1) * 512],
                                    start=False,
                                    stop=True,
                                )
                        col = 2 * tb + h
                        junk = p1.tile([P, D], bf16, tag="actjunk")
                        nc.scalar.activation(
                            out=junk[:],
                            in_=kps[:, :],
                            func=AF.Square,
                            accum_out=kk_all[:, col: col + 1],
                        )
                        qsrc = hid_b[:, h * D:(h + 1) * D]
                        if has_wqk:
                            hw = p1.tile([P, D], f32, tag="hidw")
                            nc.vector.tensor_tensor(
                                out=hw[:], in0=qsrc, in1=wqk_sb[:, h * D:(h + 1) * D],
                                op=OP.mult,
                            )
                            qk_in0 = hw[:]
                        else:
                            qk_in0 = qsrc
                        nc.vector.scalar_tensor_tensor(
                            out=dve_dummy[:].broadcast_to([P, D]),
                            in0=qk_in0,
                            scalar=1.0,
                            in1=kps[:, :],
                            op0=OP.bypass,
                            op1=OP.mult,
                            accum_out=qk_all[:, col: col + 1],
                        )
                        junk2 = p1.tile([P, D], bf16, tag="actjunk2")
                        nc.scalar.activation(
                            out=junk2[:],
                            in_=qsrc,
                            func=AF.Square,
                            accum_out=qq_all[:, col: col + 1],
                        )

            # ================= BULK GATE =================
            with ExitStack() as c2:
                bg = c2.enter_context(tc.tile_pool(name="bg", bufs=1))
                bgps = c2.enter_context(tc.tile_pool(name="bgps", bufs=1, space="PSUM"))
                DE = float(D) * EPS
                aq = bg.tile([P, NCOL], f32)
                # (qq+De)/1024  -> folds the sqrt(D) scale into the denom
                nc.vector.tensor_scalar(aq[:], qq_all[:], DE, 1.0 / D, OP.add, OP.mult)
                ak = bg.tile([P, NCOL], f32)
                nc.vector.tensor_scalar(ak[:], kk_all[:], DE, None, OP.add)
                u = bg.tile([P, NCOL], f32)
                nc.vector.tensor_tensor(out=u[:], in0=aq[:], in1=ak[:], op=OP.mult)
                sden = bg.tile([P, NCOL], f32)
                nc.scalar.activation(sden[:], u[:], AF.Sqrt)
                rden = bg.tile([P, NCOL], f32)
                nc.vector.reciprocal(rden[:], sden[:])
                s = bg.tile([P, NCOL], f32)
                nc.vector.tensor_tensor(out=s[:], in0=qk_all[:], in1=rden[:], op=OP.mult)
                s2 = bg.tile([P, NCOL], f32)
                nc.vector.tensor_tensor(out=s2[:], in0=s[:], in1=s[:], op=OP.mult)
                nc.vector.tensor_scalar_max(s2[:], s2[:], 1e-12)
                sab = bg.tile([P, NCOL], f32)
                nc.scalar.activation(sab[:], s2[:], AF.Sqrt)      # = max(|s|,1e-6)
                q4 = bg.tile([P, NCOL], f32)
                nc.scalar.activation(q4[:], sab[:], AF.Sqrt)      # = |s|_c^(1/2)
                rq4 = bg.tile([P, NCOL], f32)
                nc.vector.reciprocal(rq4[:], q4[:])
                z = bg.tile([P, NCOL], f32)
                nc.vector.tensor_tensor(out=z[:], in0=s[:], in1=rq4[:], op=OP.mult)
                th = bg.tile([P, NCOL], f32)
                nc.scalar.activation(th[:], z[:], AF.Tanh, scale=0.5)
                gate = bg.tile([P, NCOL], f32)
                nc.vector.tensor_scalar(gate[:], th[:], 0.5, 0.5, OP.mult, OP.add)
                # transpose gate [P, NCOL] -> [NCOL, P] and lay out per head
                gate_r = bg.tile([P, P], f32r)
                nc.vector.memset(gate_r[:].bitcast(f32), 0.0)
                nc.vector.tensor_copy(gate_r[:, 0:NCOL], gate[:])
                gps = bgps.tile([P, P], f32r, space="PSUM")
                nc.tensor.transpose(
                    out=gps[:, :], in_=gate_r[:, :], identity=ident_r[:]
                )
                gT = bg.tile([NCOL, P], f32)
                nc.vector.tensor_copy(gT[:, :], gps[0:NCOL, :])
                nc.sync.dma_start(gstage_d[:, :], gT[:, :])
                for h in range(HC):
                    glin = bg.tile([1, T], f32, tag=f"glin{h}")
                    nc.sync.dma_start(
                        glin[0:1, :].rearrange("h (t p) -> h t p", p=P),
                        gstage_d[:, :].rearrange("(t h) p -> h t p", h=HC)[
                            h: h + 1, :, :
                        ],
                    )
                    nc.gpsimd.partition_broadcast(gate_bc[:, h, :], glin[0:1, :])

            # ================= PASS 2 =================
            for half in range(2):
                g0 = half * cfg.half_out  # first gated token col of this half
                with ExitStack() as c3:
                    p2 = c3.enter_context(tc.tile_pool(name=f"p2_{half}", bufs=1))
                    accp = c3.enter_context(tc.tile_pool(name=f"acc_{half}", bufs=2))
                    p2ps = c3.enter_context(
                        tc.tile_pool(name=f"p2ps_{half}", bufs=3, space="PSUM")
                    )
                    outps = c3.enter_context(
                        tc.tile_pool(name=f"ops_{half}", bufs=2, space="PSUM")
                    )
                    HCOL = cfg.half_cols
                    gated = p2.tile([P, HC, 8, HCOL], bf16)

                    def mem_window(k, t0, wN):
                        # e-chunk k of mem_T over tokens [t0, t0+wN); t0 is
                        # always 128-aligned, wN is a multiple of 128 or fits
                        # within one tile.
                        assert t0 % P == 0
                        tb0 = t0 // P
                        if wN % P == 0:
                            return mem_T[:, tb0: tb0 + wN // P, k, :]
                        assert wN < P
                        return mem_T[:, tb0, k, 0:wN]

                    for dc in range(8):
                        wof = 0
                        for wN in cfg.half_windows:
                            vps = p2ps.tile([P, 512], f32, tag="vps", space="PSUM")
                            # token columns g0+wof .. g0+wof+wN of mem_T
                            t0 = g0 + wof
                            for k in range(4):
                                nc.tensor.matmul(
                                    out=vps[:, 0:wN],
                                    lhsT=vw_r[:, k, dc * P:(dc + 1) * P],
                                    rhs=mem_window(k, t0, wN),
                                    start=(k == 0),
                                    stop=(k == 3) and not has_vb,
                                )
                            if has_vb:
                                nc.tensor.matmul(
                                    out=vps[:, 0:wN],
                                    lhsT=vb_r[:, dc * P:(dc + 1) * P],
                                    rhs=ones1_r[:, 0:wN],
                                    start=False,
                                    stop=True,
                                )
                            for h in range(HC):
                                nc.vector.tensor_tensor(
                                    out=gated[:, h, dc, wof: wof + wN],
                                    in0=vps[:, 0:wN],
                                    in1=gate_bc[:, h, t0: t0 + wN],
                                    op=OP.mult,
                                )
                            wof += wN
                    # conv + silu + residual per channel chunk
                    HO = cfg.half_out
                    ocs = []
                    for h in range(HC):
                        for dc in range(8):
                            cc = h * 8 + dc
                            g = gated[:, h, dc, :]
                            a1 = accp.tile([P, HO], bf16, tag="acc")
                            nc.vector.tensor_scalar_mul(
                                a1[:], g[0:P, 0:HO], wconv_sb[:, cc, 0:1]
                            )
                            prev = a1
                            for j in range(1, K):
                                aj = accp.tile([P, HO], bf16, tag="acc")
                                nc.vector.scalar_tensor_tensor(
                                    out=aj[:],
                                    in0=g[0:P, j: j + HO],
                                    scalar=wconv_sb[:, cc, j: j + 1],
                                    in1=prev[:],
                                    op0=OP.mult,
                                    op1=OP.add,
                                )
                                prev = aj
                            ysil = accp.tile([P, HO], bf16, tag="ysil")
                            nc.scalar.activation(
                                out=ysil[:],
                                in_=prev[:],
                                func=AF.Silu,
                                bias=cb_sb[:, cc: cc + 1],
                            )
                            oc = p2.tile([P, HO], bf16, tag=f"oc{cc}")
                            nc.vector.tensor_tensor(
                                out=oc[:], in0=g[0:P, HALO: HALO + HO],
                                in1=ysil[:], op=OP.add,
                            )
                            ocs.append(oc)
                    # transpose back to token-major + store, per token block
                    for m in range(cfg.n_blk):
                        ops_t = outps.tile([P, 16, P], bf16, tag="outps",
                                           space="PSUM")
                        for cc in range(16):
                            nc.tensor.transpose(
                                out=ops_t[:, cc, :],
                                in_=ocs[cc][:, m * P:(m + 1) * P],
                                identity=ident_b[:],
                            )
                        osb = accp.tile([P, C], bf16, tag="osb")
                        nc.vector.tensor_copy(osb[:], ops_t[:, :, :])
                        r0 = half * cfg.half_out + m * P
                        nc.gpsimd.dma_start(out_d[r0: r0 + P, :], osb[:])

    nc.compile()
    return nc


def _prep_core_inputs(cfg, core, flat_ids, hid_flat, shared):
    """Slice per-core inputs. flat_ids [B, L, NH] i32, hid_flat [B, L, C] f32."""
    T, NT = cfg.T, cfg.n_tiles
    b = core // 2
    half = core % 2
    start = half * cfg.out_tok
    ZID = VOCAB * NH  # zero row

    ids_core = np.full((T, NH), ZID, dtype=np.int32)
    hid_core = np.zeros((T, C), dtype=np.float32)
    ids_core[HALO: HALO + cfg.out_tok] = flat_ids[b, start: start + cfg.out_tok]
    hid_core[HALO: HALO + cfg.out_tok] = hid_flat[b, start: start + cfg.out_tok]
    if start >= HALO:
        ids_core[0:HALO] = flat_ids[b, start - HALO: start]
        hid_core[0:HALO] = hid_flat[b, start - HALO: start]

    idsb = np.ascontiguousarray(
        ids_core.reshape(NT, P, NH).transpose(1, 0, 2).reshape(P, NT * NH)
    )
    m = dict(shared)
    m["hid"] = hid_core
    m["idsb"] = idsb
    return m


def kernel(hidden_states, hash_ids, offsets, emb_table, value_w, value_b,
           key_w, key_b, norm_key_w, norm_query_w, conv_w, conv_b):
    cfg = Cfg(full=True)
    return _run(cfg, hidden_states, hash_ids, offsets, emb_table, value_w,
                value_b, key_w, key_b, norm_key_w, norm_query_w, conv_w,
                conv_b)


def _run(cfg, hidden_states, hash_ids, offsets, emb_table, value_w, value_b,
         key_w, key_b, norm_key_w, norm_query_w, conv_w, conv_b,
         trace=False):
    hidden_states = np.asarray(hidden_states)
    hash_ids = np.asarray(hash_ids, dtype=np.int32)
    offsets = np.asarray(offsets, dtype=np.int32)
    emb_table = np.asarray(emb_table, dtype=np.float32)

    wqk = (np.asarray(norm_query_w) * np.asarray(norm_key_w)).astype(np.float32)
    has_wqk = not np.allclose(wqk, 1.0)
    has_kb = not np.all(np.asarray(key_b) == 0.0)
    has_vb = not np.all(np.asarray(value_b) == 0.0)

    key = (cfg.n_tiles, has_kb, has_vb, has_wqk)
    if key not in _CACHE:
        _CACHE[key] = _build(cfg, has_kb, has_vb, has_wqk)
    nc = _CACHE[key]

    # shared (replicated) input arrays
    emb_aug = np.zeros((VOCAB * NH + 8, ED), dtype=np.float32)
    emb_aug[: VOCAB * NH] = emb_table
    vw_h = np.ascontiguousarray(
        np.asarray(value_w, dtype=np.float32).reshape(4, P, D).transpose(1, 0, 2)
    )
    kw_h = np.ascontiguousarray(
        np.asarray(key_w, dtype=np.float32)
        .reshape(HC, 4, P, D)
        .transpose(2, 0, 1, 3)
    )
    wc = np.asarray(conv_w, dtype=np.float32)[:, 0, :]  # [C, K]
    wconv_h = np.ascontiguousarray(wc.reshape(16, P, K).transpose(1, 0, 2))
    cb_h = np.ascontiguousarray(
        np.asarray(conv_b, dtype=np.float32).reshape(16, P).T
    )
    shared = dict(emb=emb_aug, vw=vw_h, kw=kw_h, wconv=wconv_h, cb=cb_h)
    if has_wqk:
        shared["wqkb"] = np.ascontiguousarray(
            np.broadcast_to(wqk.reshape(1, C), (P, C))
        ).astype(np.float32)
    if has_kb:
        shared["kb"] = np.asarray(key_b, dtype=np.float32).reshape(1, HC, D)
    if has_vb:
        shared["vb"] = np.asarray(value_b, dtype=np.float32).reshape(1, D)

    flat_ids = hash_ids + offsets[None, None, :]
    hid_flat = np.ascontiguousarray(
        hidden_states.reshape(B, L, C), dtype=np.float32
    )

    in_maps = [
        _prep_core_inputs(cfg, c, flat_ids, hid_flat, shared)
        for c in range(N_CORES)
    ]
    res = run_bass_kernel_spmd(
        nc, in_maps, core_ids=list(range(N_CORES)), trace=trace
    )

    out = np.empty((B, L, HC, D), dtype=np.float32)
    for c in range(N_CORES):
        b, half = c // 2, c % 2
        start = half * cfg.out_tok
        out[b, start: start + cfg.out_tok] = res.results[c]["out"].reshape(
            cfg.out_tok, HC, D
        )
    if trace:
        return out, res
    return out
